# revision 51
# baseline (speedup 1.0000x reference)
"""GAT message-passing kernel for trn2, 8-core SPMD.

Strategy:
- Nodes sharded contiguously across 8 cores (6250/core, padded to 6272 = 49*128).
- Node phase: one fused matmul per 128-node window with rhs =
  [W_src | W_src@A_src | W_dst@A_dst] (host-precomputed) yields the message
  features xs AND both attention dot-products (s_src, s_dst) in one PSUM pass.
- Per layer each core AllGathers a [N, 264B] table of (xs fp8, s_src fp16)
  rows, then processes the edges whose DESTINATION is local: edges sorted by
  dst, grouped into 128-dst windows, 128-edge tiles. Per tile one indirect-DMA
  gathers the 128 source rows (994ns SWDGE fixed cost each — the dominant
  engine cost; batched offsets and dma_gather were both measured slower);
  a host-precomputed 0/1 mask matmul does the segment-reduce into a PSUM
  accumulator [128 dst, 260] (weighted features + softmax denominator), and a
  transposed mask matmul expands per-dst scores to edges.
- The AllGather is chunked (6 chunks of windows, chunk-major table row ids) and
  software-pipelined: layer l+1's node phase + AG chunks are emitted between
  layer l's edge-phase chunks. Within each window, edges are ordered by the AG
  chunk their source row lands in, and each tile's gather uses a prefix-slice
  source AP — so early tiles only depend on early AG chunks and the edge phase
  starts as soon as chunk 0 arrives, hiding nearly all collective latency.
- Softmax without max-subtraction (scores are O(1), exp is safe; ratio is
  mathematically identical); w = exp(leaky(u)) = max(exp(u), exp(0.2u)).
- Graph pooling via per-window matmul against a host-built (1/cnt)-valued
  selection matrix; partials AllReduced; the tiny 3-layer MLP runs replicated.
"""
import sys
sys.path.insert(0, '/opt/trn_rl_repo')

import numpy as np
import concourse.bass as bass
from concourse import bacc
import concourse.mybir as mybir
import concourse.tile as tile

F32 = mybir.dt.float32
F16 = mybir.dt.float16
F8 = mybir.dt.float8e4
I32 = mybir.dt.int32
AF = mybir.ActivationFunctionType
OP = mybir.AluOpType
AX = mybir.AxisListType

H, C = 4, 64
D = H * C          # 256
LAYERS = 3
NCORES = 8
LEAKY = 0.2
TBL = D + 4        # 260: xs(256) + s_src(4) node-phase psum cols
FP8_TABLE = True   # xs features stored fp8 in the gathered table
ROWB = (D + 2 * H) if FP8_TABLE else (2 * D + 2 * H)  # table row bytes
ROWW = ROWB // 4   # f32 words per table row
ROWH = ROWB // 2   # f16 words per table row


# ---------------------------------------------------------------- host side

def build_meta(x, edge_attr, edge_index, batch, n_cores=NCORES, min_G=1):
    N, FIN = x.shape
    E = edge_index.shape[1]
    G = max(int(batch.max()) + 1 if batch.size else 1, min_G)
    assert N % n_cores == 0
    Nd = N // n_cores
    W = (Nd + 127) // 128          # windows per core
    Ndp = W * 128                  # padded nodes per core

    row = np.asarray(edge_index[0], dtype=np.int64)
    col = np.asarray(edge_index[1], dtype=np.int64)
    ea = np.asarray(edge_attr, dtype=np.float32)
    batch = np.asarray(batch, dtype=np.int64)

    # global padded row id used for the gather (AG output has per-core pad rows)
    def gpad(r):
        return (r // Nd) * Ndp + (r % Nd)

    NCHUNK = 6
    cb = np.linspace(0, W, NCHUNK + 1).astype(np.int64)   # chunk window bounds
    chunks = [(int(cb[i]), int(cb[i + 1])) for i in range(NCHUNK)]
    # chunk-major AG row layout: chunk k holds cores' rows for windows [w0,w1)
    base8 = np.zeros(NCHUNK + 1, dtype=np.int64)
    for k, (w0, w1) in enumerate(chunks):
        base8[k + 1] = base8[k] + 8 * (w1 - w0) * 128
    chunk_of = np.zeros(W, dtype=np.int64)
    for k, (w0, w1) in enumerate(chunks):
        chunk_of[w0:w1] = k

    def gpad2(r):
        c = r // Nd
        n = r % Nd
        w = n // 128
        p = n % 128
        k = chunk_of[w]
        w0 = cb[k]
        rows_k = (cb[k + 1] - w0) * 128
        return base8[k] + c * rows_k + (w - w0) * 128 + p

    cores = []
    # per (core, window) edge lists; within a window order edges by the AG
    # chunk their SOURCE row lands in, so early tiles only depend on early
    # AllGather chunks (finer gather->collective dependencies)
    for c in range(n_cores):
        sel = (col >= c * Nd) & (col < (c + 1) * Nd)
        r_c = row[sel]
        l_c = col[sel] - c * Nd
        e_c = ea[sel]
        k_c = chunk_of[(r_c % Nd) // 128]
        order = np.argsort((l_c // 128) * NCHUNK + k_c, kind='stable')
        r_c, l_c, e_c, k_c = r_c[order], l_c[order], e_c[order], k_c[order]
        w_c = l_c // 128
        starts = np.searchsorted(w_c, np.arange(W + 1))
        cores.append((r_c, l_c, e_c, k_c, starts))

    # common schedule: tiles per window = max over cores
    tiles_w = np.zeros(W, dtype=np.int64)
    for wdx in range(W):
        mx = 1
        for c in range(n_cores):
            starts = cores[c][4]
            ne = starts[wdx + 1] - starts[wdx]
            mx = max(mx, (ne + 127) // 128)
        tiles_w[wdx] = mx
    T = int(tiles_w.sum())
    tile_w0 = np.concatenate([[0], np.cumsum(tiles_w)])  # tile offset per window

    idx_all = np.zeros((n_cores, 128, T), dtype=np.int32)
    mask_all = np.zeros((n_cores, 128, T * 128), dtype=np.float16)
    maskT_all = np.zeros((n_cores, 128, T * 128), dtype=np.float16)
    ea_all = np.zeros((n_cores, 128, T * 2), dtype=np.float16)
    need_all = np.zeros((n_cores, T), dtype=np.int64)

    for c in range(n_cores):
        r_c, l_c, e_c, k_c, starts = cores[c]
        for wdx in range(W):
            s, e = starts[wdx], starts[wdx + 1]
            ne = e - s
            nt = int(tiles_w[wdx])
            cap = nt * 128
            rr = np.zeros(cap, dtype=np.int64)
            ll = np.full(cap, wdx * 128, dtype=np.int64)   # pad dst -> window base
            eaw = np.zeros((cap, 2), dtype=np.float32)
            valid = np.zeros(cap, dtype=bool)
            kk = np.zeros(cap, dtype=np.int64)
            rr[:ne] = r_c[s:e]
            ll[:ne] = l_c[s:e]
            eaw[:ne] = e_c[s:e]
            valid[:ne] = True
            kk[:ne] = k_c[s:e]
            slot = ll - wdx * 128
            t0 = tile_w0[wdx]
            for t in range(nt):
                sl = slice(t * 128, (t + 1) * 128)
                idx_all[c, :, t0 + t] = gpad2(rr[sl]).astype(np.int32)
                need_all[c, t0 + t] = kk[sl].max() if np.any(valid[sl]) else 0
                m = np.zeros((128, 128), dtype=np.float16)
                vv = valid[sl]
                m[np.arange(128)[vv], slot[sl][vv]] = np.float16(1.0)
                mask_all[c, :, (t0 + t) * 128:(t0 + t + 1) * 128] = m
                maskT_all[c, :, (t0 + t) * 128:(t0 + t + 1) * 128] = m.T
                ea_all[c, :, (t0 + t) * 2:(t0 + t + 1) * 2] = eaw[sl].astype(np.float16)

    # pooling selection: [128, W*G] value 1/cnt
    cnt = np.bincount(batch, minlength=G).astype(np.float64)
    cnt = np.maximum(cnt, 1.0)
    pool_all = np.zeros((n_cores, 128, W * G), dtype=np.float32)
    for c in range(n_cores):
        for wdx in range(W):
            base = c * Nd + wdx * 128
            nn = min(128, Nd - wdx * 128)
            if nn <= 0:
                continue
            gs = batch[base:base + nn]
            pool_all[c, np.arange(nn), wdx * G + gs] = (1.0 / cnt[gs])
    pool_all = pool_all.astype(np.float16)

    # x transposed + padded per core: [FIN, Ndp]
    xT = np.zeros((n_cores, FIN, Ndp), dtype=np.float16)
    for c in range(n_cores):
        xT[c, :, :Nd] = np.asarray(x[c * Nd:(c + 1) * Nd], dtype=np.float32).T.astype(np.float16)

    return dict(N=N, FIN=FIN, E=E, G=G, Nd=Nd, Ndp=Ndp, W=W, T=T,
                tiles_w=tiles_w.tolist(), tile_w0=tile_w0.tolist(),
                chunks=chunks, need=need_all.max(axis=0).tolist(),
                n_cores=n_cores, idx=idx_all, mask=mask_all,
                maskT=maskT_all, ea=ea_all, pool=pool_all, xT=xT)


def _const_flags(params):
    f = {}
    f['b_pre0'] = not np.any(params['b_pre'])
    f['bias_conv0'] = not np.any(params['bias_conv'])
    f['gamma1'] = bool(np.all(params['ln_gamma'] == 1.0))
    f['beta0'] = not np.any(params['ln_beta'])
    pa = params['prelu_a']
    f['prelu_const'] = bool(np.all(pa == pa.flat[0])) and 0.0 <= float(pa.flat[0]) <= 1.0
    f['prelu_val'] = float(pa.flat[0])
    f['b_post1_0'] = not np.any(params['b_post1'])
    f['b_post2_0'] = not np.any(params['b_post2'])
    f['b_risk0'] = not np.any(params['b_risk'])
    return f


# ---------------------------------------------------------------- program

def build_program(meta, flags, dbg=False, variant=None):
    n_cores = meta['n_cores']
    G, W, T, Ndp, FIN = meta['G'], meta['W'], meta['T'], meta['Ndp'], meta['FIN']
    tiles_w, tile_w0 = meta['tiles_w'], meta['tile_w0']
    need = meta['need']
    Tmax = max(tiles_w)
    KF = FIN // 128   # k-tiles for input features
    assert D % 128 == 0
    KD = D // 128     # 2

    vset = set((variant or '').split('+')) if variant else set()
    skip_ag = bool(vset & {'noag', 'sim1'})
    skip_gather = 'nogather' in vset
    skip_mask = 'nomask' in vset
    skip_edge = 'noedge' in vset
    tilegather = 'tilegather' in vset
    q4 = 'q4' in vset
    ndev = 1 if 'sim1' in vset else n_cores

    nc = bacc.Bacc('TRN2', target_bir_lowering=False, debug=False,
                   num_devices=ndev, num_swdge_queues=4 if q4 else 1)

    # ---- dram inputs
    d_xT = nc.dram_tensor("xT", [FIN, Ndp], F16, kind="ExternalInput")
    d_idx = nc.dram_tensor("idx", [128, T], I32, kind="ExternalInput")
    d_mask = nc.dram_tensor("mask", [128, T * 128], F16, kind="ExternalInput")
    d_maskT = nc.dram_tensor("maskT", [128, T * 128], F16, kind="ExternalInput")
    d_ea = nc.dram_tensor("ea", [128, T * 2], F16, kind="ExternalInput")
    d_pool = nc.dram_tensor("pool", [128, W * G], F16, kind="ExternalInput")
    d_wpre = nc.dram_tensor("w_pre", [FIN, D], F16, kind="ExternalInput")
    d_bpre = nc.dram_tensor("b_pre", [1, D], F32, kind="ExternalInput")
    # w_src extended with fused att columns: [D, D+2H] = [W_src | W_src@A_src
    # | W_dst@A_dst] — one matmul chain yields xs, s_src and s_dst at once
    d_wsrc = nc.dram_tensor("w_src", [LAYERS, D, TBL + H], F16, kind="ExternalInput")
    d_wedge = nc.dram_tensor("w_edge", [LAYERS, 2, D], F32, kind="ExternalInput")
    d_aedge = nc.dram_tensor("att_edge", [LAYERS, 1, D], F32, kind="ExternalInput")
    d_bconv = nc.dram_tensor("bias_conv", [LAYERS, 1, D], F32, kind="ExternalInput")
    d_gamma = nc.dram_tensor("ln_gamma", [LAYERS, 1, D], F32, kind="ExternalInput")
    d_beta = nc.dram_tensor("ln_beta", [LAYERS, 1, D], F32, kind="ExternalInput")
    d_prelu = nc.dram_tensor("prelu_a", [LAYERS, 1, D], F32, kind="ExternalInput")
    d_w1 = nc.dram_tensor("w_post1", [D * (LAYERS + 1), D], F16, kind="ExternalInput")
    d_b1 = nc.dram_tensor("b_post1", [1, D], F32, kind="ExternalInput")
    d_w2 = nc.dram_tensor("w_post2", [D, D], F16, kind="ExternalInput")
    d_b2 = nc.dram_tensor("b_post2", [1, D], F32, kind="ExternalInput")
    d_wr = nc.dram_tensor("w_risk", [D, 1], F16, kind="ExternalInput")
    d_br = nc.dram_tensor("b_risk", [1, 1], F32, kind="ExternalInput")
    d_out = nc.dram_tensor("risk", [G, 1], F32, kind="ExternalOutput")
    if dbg:
        d_dbg_h0 = nc.dram_tensor("dbg_h0", [128, D], F32, kind="ExternalOutput")
        d_dbg_tbl = nc.dram_tensor("dbg_tbl", [256, ROWW], F32, kind="ExternalOutput")
        d_dbg_g = nc.dram_tensor("dbg_g", [128, ROWW], F32, kind="ExternalOutput")
        d_dbg_u = nc.dram_tensor("dbg_u", [128, TBL], F32, kind="ExternalOutput")
        d_dbg_t16 = nc.dram_tensor("dbg_t16", [128, D], F32, kind="ExternalOutput")
        d_dbg_pool = nc.dram_tensor("dbg_pool", [32, (LAYERS + 1) * D], F32, kind="ExternalOutput")
        d_dbg_xc = nc.dram_tensor("dbg_xc", [32, (LAYERS + 1) * D], F32, kind="ExternalOutput")
        d_dbg_p1 = nc.dram_tensor("dbg_p1", [32, D], F32, kind="ExternalOutput")
        d_dbg_xct = nc.dram_tensor("dbg_xct", [128, (D * (LAYERS + 1) // 128) * 32], F16, kind="ExternalOutput")
        d_dbg_p2 = nc.dram_tensor("dbg_p2", [32, D], F32, kind="ExternalOutput")

    from contextlib import ExitStack
    with tile.TileContext(nc) as tc, ExitStack() as _st:
        if True:
            pp = _st.enter_context(tc.tile_pool(name="persist", bufs=1))
            wrep = _st.enter_context(tc.tile_pool(name="wrep", bufs=2))
            wts = _st.enter_context(tc.tile_pool(name="wts", bufs=2))
            ttp = _st.enter_context(tc.tile_pool(name="ttile", bufs=3))
            hTp = _st.enter_context(tc.tile_pool(name="hT", bufs=2))
            gp = _st.enter_context(tc.tile_pool(name="gath", bufs=6))
            mp = _st.enter_context(tc.tile_pool(name="maskp", bufs=3))
            mtp = _st.enter_context(tc.tile_pool(name="maskTp", bufs=3))
            mpr = _st.enter_context(tc.tile_pool(name="mprime", bufs=3))
            scr = _st.enter_context(tc.tile_pool(name="scr", bufs=4))
            scr1 = _st.enter_context(tc.tile_pool(name="scr1", bufs=4))
            sdstp = _st.enter_context(tc.tile_pool(name="sdstp", bufs=2))
            sep = _st.enter_context(tc.tile_pool(name="sep", bufs=2))
            ps_u = _st.enter_context(tc.tile_pool(name="ps_u", bufs=3, space="PSUM"))
            ps_sb = _st.enter_context(tc.tile_pool(name="ps_sb", bufs=1, space="PSUM"))
            ps_n = _st.enter_context(tc.tile_pool(name="ps_n", bufs=2, space="PSUM"))
            ps_pool = _st.enter_context(tc.tile_pool(name="ps_pool", bufs=1, space="PSUM"))
            ps_t = _st.enter_context(tc.tile_pool(name="ps_t", bufs=1, space="PSUM"))
            dp = _st.enter_context(tc.tile_pool(name="dram", bufs=3, space="DRAM"))
            # ---------------- persistent tiles
            h_sb = pp.tile([128, W, D], F16, tag="h")
            pass  # t16 allocated after xT16 (shared tag)
            idx_sb = pp.tile([128, T], I32, tag="idx")
            ea_sb = pp.tile([128, T, 2], F16, tag="ea")
            pool_sb = pp.tile([128, W, G], F16, tag="pool")
            pooled_sb = pp.tile([32, LAYERS + 1, D], F32, tag="pooled")
            mstat = pp.tile([128, W], F32, tag="mstat")
            sstat = pp.tile([128, W], F32, tag="sstat")
            rstd_t = pp.tile([128, W], F32, tag="rstd")
            nmrs_t = pp.tile([128, W], F32, tag="nmrs")
            ident = pp.tile([128, 128], F16, tag="ident")
            ones1 = pp.tile([1, 128], F32, tag="ones1")

            from concourse.masks import make_identity
            make_identity(nc, ident[:])
            nc.gpsimd.memset(pooled_sb[:], 0.0)
            nc.gpsimd.memset(ones1[:], 1.0)

            g_zero = msk_zero = mskT_zero = None
            if skip_gather:
                g_zero = pp.tile([128, Tmax, ROWW], F32, tag="gzero")
                nc.gpsimd.memset(g_zero[:], 0.0)
            if skip_mask:
                msk_zero = pp.tile([128, Tmax * 128], F16, tag="mzero")
                mskT_zero = pp.tile([128, Tmax * 128], F16, tag="mtzero")
                nc.gpsimd.memset(msk_zero[:], 0.0)
                nc.gpsimd.memset(mskT_zero[:], 0.0)

            nc.sync.dma_start(out=idx_sb[:], in_=d_idx[:])
            nc.sync.dma_start(out=ea_sb[:], in_=d_ea[:].rearrange("p (t k) -> p t k", k=2))
            nc.sync.dma_start(out=pool_sb[:], in_=d_pool[:].rearrange("p (w g) -> p w g", g=G))

            def bcast_load(pool_, dram_ap, parts, width, dt=F32, tag=None):
                t = pool_.tile([parts, width], dt, tag=tag or "bc")
                nc.sync.dma_start(out=t[:], in_=dram_ap.to_broadcast([parts, width]))
                return t

            # ---------------- pre phase + software-pipelined layers
            CHUNKS = meta['chunks']
            NCH = len(CHUNKS)
            base8 = [0]
            for (w0, w1) in CHUNKS:
                base8.append(base8[-1] + 8 * (w1 - w0) * 128)

            wpre16 = wts.tile([128, KF, D], F16, tag="wmat")
            nc.gpsimd.dma_start(out=wpre16[:], in_=d_wpre[:].rearrange("(k p) d -> p k d", p=128))
            bpre_rep = None
            if not flags['b_pre0']:
                bpre_rep = bcast_load(wrep, d_bpre[:], 128, D, tag="bpre")
            xT16 = pp.tile([128, KF, Ndp], F16, tag="big")
            nc.gpsimd.dma_start(out=xT16[:], in_=d_xT[:].rearrange("(k p) n -> p k n", p=128))
            t16_sb = pp.tile([128, W, D], F16, tag="big")

            ws16_of, sdst_of, se_of, tsh_of = {}, {}, {}, {}

            def prep_layer(l):
                ws16 = wts.tile([128, KD, TBL + H], F16, tag="wmat")
                nc.gpsimd.dma_start(out=ws16[:], in_=d_wsrc[l].rearrange("(k p) d -> p k d", p=128))
                ws16_of[l] = ws16
                sdst_t = sdstp.tile([128, W * H], F16, tag="sdst16")
                sdst_of[l] = sdst_t
                # q for edge scores: [2, H], replicated to 128 partitions
                aedge_rep = bcast_load(scr1, d_aedge[l], 1, D, tag="aedge")
                qrep = scr1.tile([128, 2, H], F32, tag="qrep")
                for k in range(2):
                    wedge_k = scr1.tile([1, D], F32, tag="wedge")
                    nc.sync.dma_start(out=wedge_k[:], in_=d_wedge[l, k:k + 1, :])
                    nc.vector.tensor_tensor(out=wedge_k[:], in0=wedge_k[:], in1=aedge_rep[:], op=OP.mult)
                    qred_k = scr1.tile([1, H], F32, tag="qred")
                    nc.vector.reduce_sum(out=qred_k[:], in_=wedge_k[:].rearrange("p (h c) -> p h c", h=H), axis=AX.X)
                    qps = ps_t.tile([128, H], F32, tag="tr")
                    nc.tensor.matmul(out=qps[:], lhsT=ones1[:], rhs=qred_k[:], start=True, stop=True)
                    nc.vector.tensor_copy(out=qrep[:, k, :], in_=qps[:])
                # se = ea0*q0 + ea1*q1  [128, T, H]
                se_t = sep.tile([128, T, H], F32, tag="se")
                tmp_se = sep.tile([128, T, H], F32, tag="tmpse")
                nc.vector.tensor_tensor(
                    out=se_t[:], in0=ea_sb[:, :, 0:1].to_broadcast([128, T, H]),
                    in1=qrep[:, 0:1, :].to_broadcast([128, T, H]), op=OP.mult)
                nc.vector.tensor_tensor(
                    out=tmp_se[:], in0=ea_sb[:, :, 1:2].to_broadcast([128, T, H]),
                    in1=qrep[:, 1:2, :].to_broadcast([128, T, H]), op=OP.mult)
                nc.vector.tensor_tensor(out=se_t[:], in0=se_t[:], in1=tmp_se[:], op=OP.add)
                se_of[l] = se_t
                if skip_ag:
                    tsh_of[l] = nc.dram_tensor(f"tshared_l{l}", [n_cores * Ndp, ROWW], F32)
                else:
                    tsh_of[l] = nc.dram_tensor(f"tshared_l{l}", [n_cores * Ndp, ROWW], F32,
                                               addr_space="Shared")

            def node_window(l, w, tb_t, wk0):
                ws16 = ws16_of[l]
                hTw = hTp.tile([128, KD, 128], F16, tag="hT")
                for k in range(KD):
                    tps = ps_t.tile([128, 128], F16, tag="tr")
                    nc.tensor.transpose(out=tps[:], in_=h_sb[:, w, k * 128:(k + 1) * 128], identity=ident[:])
                    nc.scalar.activation(hTw[:, k, :], tps[:], AF.Copy)
                nx_ps = ps_n.tile([128, TBL + H], F32, tag="node")
                for k in range(KD):
                    nc.tensor.matmul(out=nx_ps[:], lhsT=hTw[:, k, :], rhs=ws16[:, k, :],
                                     start=(k == 0), stop=(k == KD - 1))
                tt = ttp.tile([128, ROWH], F16, tag="tt")
                if FP8_TABLE:
                    nc.scalar.activation(tt[:, 0:D // 2].bitcast(F8), nx_ps[:, 0:D], AF.Copy)
                    nc.vector.tensor_copy(out=tt[:, D // 2:ROWH], in_=nx_ps[:, D:TBL])
                else:
                    nc.scalar.activation(tt[:], nx_ps[:, 0:TBL], AF.Copy)
                nc.vector.tensor_copy(out=sdst_of[l][:, w * H:(w + 1) * H],
                                      in_=nx_ps[:, TBL:TBL + H])
                nc.sync.dma_start(out=tb_t[(w - wk0) * 128:(w - wk0 + 1) * 128, :],
                                  in_=tt[:].bitcast(F32))

            def ag_chunk(l, k, tb_t):
                w0, w1 = CHUNKS[k]
                rows = (w1 - w0) * 128
                tsh = tsh_of[l]
                if skip_ag:
                    nc.sync.dma_start(out=tsh[base8[k]:base8[k] + rows, :], in_=tb_t[:])
                else:
                    nc.gpsimd.collective_compute(
                        "AllGather", OP.bypass,
                        replica_groups=[list(range(n_cores))],
                        ins=[tb_t[:].opt()],
                        outs=[tsh[base8[k]:base8[k + 1], :].opt()])

            def edge_window(l, w):
                nt = tiles_w[w]
                t0 = tile_w0[w]
                tsh = tsh_of[l]
                se_t = se_of[l]
                if skip_gather:
                    g = g_zero
                else:
                    g = gp.tile([128, Tmax, ROWW], F32, tag="g")
                    for t in range(nt):
                        hi = base8[need[t0 + t] + 1]
                        nc.gpsimd.indirect_dma_start(
                            out=g[:, t, :], out_offset=None,
                            in_=tsh[0:hi, :],
                            in_offset=bass.IndirectOffsetOnAxis(ap=idx_sb[:, t0 + t:t0 + t + 1], axis=0))
                g16 = g[:].bitcast(F16)
                if FP8_TABLE:
                    xs_ap = g[:].bitcast(F8)[:, :nt, 0:D]
                    ssrc_ap = g16[:, :nt, D // 2:D // 2 + H]
                else:
                    xs_ap = g16[:, :nt, 0:D]
                    ssrc_ap = g16[:, :nt, D:D + H]
                if skip_mask:
                    msk, mskT = msk_zero, mskT_zero
                else:
                    msk = mp.tile([128, Tmax * 128], F16, tag="mask")
                    mskT = mtp.tile([128, Tmax * 128], F16, tag="maskT")
                    nc.sync.dma_start(out=msk[:, :nt * 128], in_=d_mask[:, t0 * 128:(t0 + nt) * 128])
                    nc.sync.dma_start(out=mskT[:, :nt * 128], in_=d_maskT[:, t0 * 128:(t0 + nt) * 128])
                # s_base = maskT.T @ sdst (per tile) -> [128e, H]
                sb_ps = ps_sb.tile([128, Tmax * H], F32, tag="sbase")
                for t in range(nt):
                    nc.tensor.matmul(out=sb_ps[:, t * H:(t + 1) * H],
                                     lhsT=mskT[:, t * 128:(t + 1) * 128],
                                     rhs=sdst_of[l][:, w * H:(w + 1) * H],
                                     start=True, stop=True)
                # u = s_base + s_src + s_edge
                u = scr.tile([128, Tmax, H], F32, tag="u")
                nc.vector.tensor_tensor(out=u[:, :nt, :],
                                        in0=sb_ps[:].rearrange("p (t h) -> p t h", h=H)[:, :nt, :],
                                        in1=ssrc_ap, op=OP.add)
                nc.vector.tensor_tensor(out=u[:, :nt, :], in0=u[:, :nt, :],
                                        in1=se_t[:, t0:t0 + nt, :], op=OP.add)
                # w = exp(leaky_relu(u)) = max(exp(u), exp(0.2u))
                e1 = scr.tile([128, Tmax, H], F16, tag="e1")
                e2 = scr.tile([128, Tmax, H], F16, tag="e2")
                nc.scalar.activation(e1[:, :nt, :], u[:, :nt, :], AF.Exp)
                nc.scalar.activation(e2[:, :nt, :], u[:, :nt, :], AF.Exp, scale=LEAKY)
                w16 = scr.tile([128, Tmax, H], F16, tag="w16")
                nc.vector.tensor_tensor(out=w16[:, :nt, :], in0=e1[:, :nt, :], in1=e2[:, :nt, :], op=OP.max)
                # M' = [xs * w (per head), w]
                mpr_t = mpr.tile([128, Tmax, TBL], F16, tag="mp")
                nc.vector.tensor_tensor(
                    out=mpr_t[:, :nt, 0:D].rearrange("p t (h c) -> p t h c", h=H),
                    in0=xs_ap.rearrange("p t (h c) -> p t h c", h=H),
                    in1=w16[:, :nt, :].rearrange("p t (h o) -> p t h o", o=1).to_broadcast([128, nt, H, C]),
                    op=OP.mult)
                nc.vector.tensor_copy(out=mpr_t[:, :nt, D:TBL], in_=w16[:, :nt, :])
                # U accumulation
                u_ps = ps_u.tile([128, TBL], F32, tag="U")
                for t in range(nt):
                    nc.tensor.matmul(out=u_ps[:], lhsT=msk[:, t * 128:(t + 1) * 128],
                                     rhs=mpr_t[:, t, :], start=(t == 0), stop=(t == nt - 1))
                # z-divide + stats
                zrec = scr.tile([128, H], F32, tag="zrec")
                nc.vector.tensor_scalar_add(out=zrec[:], in0=u_ps[:, D:TBL], scalar1=1e-16)
                nc.vector.reciprocal(out=zrec[:], in_=zrec[:])
                wt = scr.tile([128, D], F32, tag="wt")
                nc.vector.tensor_tensor(
                    out=wt[:].rearrange("p (h c) -> p h c", h=H),
                    in0=u_ps[:, 0:D].rearrange("p (h c) -> p h c", h=H),
                    in1=zrec[:].rearrange("p (h o) -> p h o", o=1).to_broadcast([128, H, C]),
                    op=OP.mult)
                if not flags['bias_conv0']:
                    bconv_rep = bcast_load(wrep, d_bconv[l], 128, D, tag="bconv")
                    nc.vector.tensor_tensor(out=wt[:], in0=wt[:], in1=bconv_rep[:], op=OP.add)
                # mean/sq-sum stats + fp16 copy
                nc.scalar.activation(t16_sb[:, w, :], wt[:], AF.Copy,
                                     accum_out=mstat[:, w:w + 1])
                sqt = scr.tile([128, D], F16, tag="sqt")
                nc.scalar.activation(sqt[:], t16_sb[:, w, :], AF.Square,
                                     accum_out=sstat[:, w:w + 1])

            # ----- pre phase (+ layer-0 node phase & chunked AG interleaved)
            prep_layer(0)
            x0_ps = ps_pool.tile([32, D], F32, tag="gpool")
            for k, (w0, w1) in enumerate(CHUNKS):
                for w in range(w0, w1):
                    hps = ps_n.tile([128, D], F32, tag="node")
                    for kk in range(KF):
                        nc.tensor.matmul(out=hps[:], lhsT=xT16[:, kk, w * 128:(w + 1) * 128],
                                         rhs=wpre16[:, kk, :], start=(kk == 0), stop=(kk == KF - 1))
                    if bpre_rep is not None:
                        nc.vector.tensor_tensor(out=hps[:], in0=hps[:], in1=bpre_rep[:], op=OP.add)
                    nc.scalar.activation(h_sb[:, w, :], hps[:], AF.Copy)
                    nc.tensor.matmul(out=x0_ps[:G, :], lhsT=pool_sb[:, w, :], rhs=h_sb[:, w, :],
                                     start=(w == 0), stop=(w == W - 1))
                tb_t = dp.tile([(w1 - w0) * 128, ROWW], F32, tag="tb")
                for w in range(w0, w1):
                    node_window(0, w, tb_t, w0)
                ag_chunk(0, k, tb_t)
            nc.vector.tensor_copy(out=pooled_sb[:G, 0, :], in_=x0_ps[:G, :])

            # ----- layers: edge(l) with node(l+1)+AG(l+1) interleaved per chunk
            for l in range(LAYERS):
                lpool_ps = ps_pool.tile([32, D], F32, tag="gpool")
                gamma_rep = beta_rep = prelua_rep = None
                if not flags['gamma1']:
                    gamma_rep = bcast_load(wrep, d_gamma[l], 128, D, tag="gamma")
                if not flags['beta0']:
                    beta_rep = bcast_load(wrep, d_beta[l], 128, D, tag="beta")
                if not flags['prelu_const']:
                    prelua_rep = bcast_load(wrep, d_prelu[l], 128, D, tag="prelua")
                last = (l == LAYERS - 1)

                def post_chunk(w0, w1):
                    sl = slice(w0, w1)
                    mean_t = scr1.tile([128, w1 - w0], F32, tag="mean")
                    nc.scalar.mul(out=mean_t[:], in_=mstat[:, sl], mul=1.0 / D)
                    var_t = scr1.tile([128, w1 - w0], F32, tag="var")
                    nc.vector.tensor_tensor(out=var_t[:], in0=mean_t[:], in1=mean_t[:], op=OP.mult)
                    nc.vector.scalar_tensor_tensor(out=var_t[:], in0=sstat[:, sl], scalar=1.0 / D,
                                                   in1=var_t[:], op0=OP.mult, op1=OP.subtract)
                    nc.vector.tensor_scalar_add(out=var_t[:], in0=var_t[:], scalar1=1e-5)
                    nc.vector.reciprocal(out=var_t[:], in_=var_t[:])
                    nc.scalar.sqrt(out=rstd_t[:, sl], in_=var_t[:])
                    nc.vector.tensor_tensor(out=nmrs_t[:, sl], in0=mean_t[:], in1=rstd_t[:, sl], op=OP.mult)
                    nc.vector.tensor_scalar_mul(out=nmrs_t[:, sl], in0=nmrs_t[:, sl], scalar1=-1.0)
                    for w in range(w0, w1):
                        y = ttp.tile([128, D], F16, tag="y")
                        if flags['prelu_const'] and gamma_rep is None and beta_rep is None:
                            nc.scalar.activation(y[:], t16_sb[:, w, :], AF.Prelu,
                                                 scale=rstd_t[:, w:w + 1],
                                                 bias=nmrs_t[:, w:w + 1],
                                                 alpha=flags['prelu_val'])
                        else:
                            nc.vector.tensor_scalar(out=y[:], in0=t16_sb[:, w, :],
                                                    scalar1=rstd_t[:, w:w + 1],
                                                    scalar2=nmrs_t[:, w:w + 1],
                                                    op0=OP.mult, op1=OP.add)
                            if gamma_rep is not None:
                                nc.vector.tensor_tensor(out=y[:], in0=y[:], in1=gamma_rep[:], op=OP.mult)
                            if beta_rep is not None:
                                nc.vector.tensor_tensor(out=y[:], in0=y[:], in1=beta_rep[:], op=OP.add)
                        if flags['prelu_const'] and gamma_rep is None and beta_rep is None:
                            pass
                        elif flags['prelu_const']:
                            nc.vector.scalar_tensor_tensor(out=y[:], in0=y[:], scalar=flags['prelu_val'],
                                                           in1=y[:], op0=OP.mult, op1=OP.max)
                        else:
                            neg = scr.tile([128, D], F32, tag="neg")
                            nc.vector.tensor_scalar_min(out=neg[:], in0=y[:], scalar1=0.0)
                            nc.vector.tensor_scalar_max(out=y[:], in0=y[:], scalar1=0.0)
                            nc.vector.scalar_tensor_tensor(out=neg[:], in0=neg[:], scalar=1.0,
                                                           in1=prelua_rep[:], op0=OP.mult, op1=OP.mult)
                            nc.vector.tensor_tensor(out=y[:], in0=y[:], in1=neg[:], op=OP.add)
                        nc.tensor.matmul(out=lpool_ps[:G, :], lhsT=pool_sb[:, w, :], rhs=y[:],
                                         start=(w == 0), stop=(w == W - 1))
                        if not last:
                            nc.vector.tensor_tensor(out=h_sb[:, w, :], in0=y[:], in1=h_sb[:, w, :], op=OP.add)

                for k, (w0, w1) in enumerate(CHUNKS):
                    if not skip_edge:
                        for w in range(w0, w1):
                            edge_window(l, w)
                        post_chunk(w0, w1)
                    if not last:
                        if k == 0:
                            prep_layer(l + 1)
                        tb_t = dp.tile([(w1 - w0) * 128, ROWW], F32, tag="tb")
                        for w in range(w0, w1):
                            node_window(l + 1, w, tb_t, w0)
                        ag_chunk(l + 1, k, tb_t)
                if not skip_edge:
                    nc.vector.tensor_copy(out=pooled_sb[:G, l + 1, :], in_=lpool_ps[:G, :])

            # ---------------- AllReduce pooled partials + MLP
            FD = (LAYERS + 1) * D     # 1024 flat features per graph
            pr_in = dp.tile([32, FD], F32, tag="prin")
            pr_out = dp.tile([32, FD], F32, tag="prout")
            nc.sync.dma_start(out=pr_in[:], in_=pooled_sb[:].rearrange("p l d -> p (l d)"))
            if skip_ag:
                nc.sync.dma_start(out=pr_out[:], in_=pr_in[:])
            else:
                nc.gpsimd.collective_compute(
                    "AllReduce", OP.add, replica_groups=[list(range(n_cores))],
                    ins=[pr_in[:].opt()], outs=[pr_out[:].opt()])
            if dbg:
                nc.sync.dma_start(out=d_dbg_pool[:], in_=pooled_sb[:].rearrange("p l d -> p (l d)"))
            K1 = FD // 128
            xcT = wts.tile([128, K1, 32], F16, tag="xcT")
            prT = pr_out[:].rearrange("g f -> f g")
            for k in range(K1):
                nc.gpsimd.dma_start(out=xcT[:, k, :], in_=prT[k * 128:(k + 1) * 128, :])
            mlp_b = dp.tile([32, D], F32, tag="mlpb")

            if dbg:
                nc.sync.dma_start(out=d_dbg_xc[:], in_=xc[:].rearrange("p l d -> p (l d)"))
            w1_sb = wts.tile([128, K1, D], F16, tag="wmlp")
            nc.gpsimd.dma_start(out=w1_sb[:], in_=d_w1[:].rearrange("(k p) d -> p k d", p=128))
            p1_ps = ps_n.tile([32, D], F32, tag="node")
            for k in range(K1):
                nc.tensor.matmul(out=p1_ps[:G, :], lhsT=xcT[:, k, :G], rhs=w1_sb[:, k, :],
                                 start=(k == 0), stop=(k == K1 - 1))
            p1 = pp.tile([32, D], F32, tag="p1")
            if not flags['b_post1_0']:
                b1_rep = bcast_load(wrep, d_b1[:], 32, D, tag="b1")
                nc.vector.tensor_tensor(out=p1_ps[:G, :], in0=p1_ps[:G, :], in1=b1_rep[:G, :], op=OP.add)
            nc.scalar.activation(p1[:G, :], p1_ps[:G, :], AF.Relu)

            if dbg:
                nc.sync.dma_start(out=d_dbg_p1[:G, :], in_=p1[:G, :])
                nc.sync.dma_start(out=d_dbg_xct[:], in_=xcT[:].rearrange("p k g -> p (k g)"))
            nc.sync.dma_start(out=mlp_b[:G, :], in_=p1[:G, :])
            p1T = wts.tile([128, KD, 32], F16, tag="p1T")
            mbT = mlp_b[:G, :].rearrange("g d -> d g")
            for k in range(KD):
                nc.gpsimd.dma_start(out=p1T[:, k, :G], in_=mbT[k * 128:(k + 1) * 128, :])
            w2_sb = wts.tile([128, KD, D], F16, tag="wmlp")
            nc.gpsimd.dma_start(out=w2_sb[:], in_=d_w2[:].rearrange("(k p) d -> p k d", p=128))
            p2_ps = ps_n.tile([32, D], F32, tag="node")
            for k in range(KD):
                nc.tensor.matmul(out=p2_ps[:G, :], lhsT=p1T[:, k, :G], rhs=w2_sb[:, k, :],
                                 start=(k == 0), stop=(k == KD - 1))
            p2 = pp.tile([32, D], F32, tag="p2")
            if not flags['b_post2_0']:
                b2_rep = bcast_load(wrep, d_b2[:], 32, D, tag="b2")
                nc.vector.tensor_tensor(out=p2_ps[:G, :], in0=p2_ps[:G, :], in1=b2_rep[:G, :], op=OP.add)
            nc.scalar.activation(p2[:G, :], p2_ps[:G, :], AF.Relu)

            if dbg:
                nc.sync.dma_start(out=d_dbg_p2[:], in_=p2[:])
            mlp_b2 = dp.tile([32, D], F32, tag="mlpb")
            nc.sync.dma_start(out=mlp_b2[:G, :], in_=p2[:G, :])
            p2T = wts.tile([128, KD, 32], F16, tag="p2T")
            mb2T = mlp_b2[:G, :].rearrange("g d -> d g")
            for k in range(KD):
                nc.gpsimd.dma_start(out=p2T[:, k, :G], in_=mb2T[k * 128:(k + 1) * 128, :])
            wr_sb = wts.tile([128, KD, 1], F16, tag="wmlp")
            nc.gpsimd.dma_start(out=wr_sb[:], in_=d_wr[:].rearrange("(k p) d -> p k d", p=128))
            r_ps = ps_n.tile([32, 1], F32, tag="node")
            for k in range(KD):
                nc.tensor.matmul(out=r_ps[:G, :], lhsT=p2T[:, k, :G], rhs=wr_sb[:, k, :],
                                 start=(k == 0), stop=(k == KD - 1))
            risk_sb = pp.tile([32, 1], F32, tag="risk")
            if not flags['b_risk0']:
                br_rep = bcast_load(wrep, d_br[:], 32, 1, tag="br")
                nc.vector.tensor_tensor(out=r_ps[:G, :], in0=r_ps[:G, :], in1=br_rep[:G, :], op=OP.add)
            nc.vector.tensor_copy(out=risk_sb[:G, :], in_=r_ps[:G, :])
            nc.sync.dma_start(out=d_out[:], in_=risk_sb[:G, :])

    nc.finalize()
    return nc


def build_in_maps(meta, params):
    n_cores = meta['n_cores']
    f32 = lambda a: np.ascontiguousarray(np.asarray(a, dtype=np.float32))
    # fold the attention dot-products into the node matmuls:
    # s_src = (h@W_src).reshape(H,C) . att_src  ==  h @ (W_src @ A_src)
    W_src = np.asarray(params['W_src'], dtype=np.float64)   # [L, D, D]
    W_dst = np.asarray(params['W_dst'], dtype=np.float64)   # [L, D, D]
    a_src = np.asarray(params['att_src'], dtype=np.float64).reshape(LAYERS, H, C)
    a_dst = np.asarray(params['att_dst'], dtype=np.float64).reshape(LAYERS, H, C)
    w_src_ext = np.zeros((LAYERS, D, TBL + H), dtype=np.float32)
    for l in range(LAYERS):
        A_s = np.zeros((D, H)); A_d = np.zeros((D, H))
        for h in range(H):
            A_s[h * C:(h + 1) * C, h] = a_src[l, h]
            A_d[h * C:(h + 1) * C, h] = a_dst[l, h]
        w_src_ext[l, :, 0:D] = W_src[l]
        w_src_ext[l, :, D:TBL] = W_src[l] @ A_s
        w_src_ext[l, :, TBL:TBL + H] = W_dst[l] @ A_d
    f16 = lambda a: np.ascontiguousarray(np.asarray(a, dtype=np.float16))
    shared = {
        "w_pre": f16(params['W_pre']),
        "b_pre": f32(params['b_pre']).reshape(1, D),
        "w_src": w_src_ext.astype(np.float16),
        "w_edge": f32(params['W_edge']),
        "att_edge": f32(params['att_edge']).reshape(LAYERS, 1, D),
        "bias_conv": f32(params['bias_conv']).reshape(LAYERS, 1, D),
        "ln_gamma": f32(params['ln_gamma']).reshape(LAYERS, 1, D),
        "ln_beta": f32(params['ln_beta']).reshape(LAYERS, 1, D),
        "prelu_a": f32(params['prelu_a']).reshape(LAYERS, 1, D),
        "w_post1": f16(params['W_post1']),
        "b_post1": f32(params['b_post1']).reshape(1, D),
        "w_post2": f16(params['W_post2']),
        "b_post2": f32(params['b_post2']).reshape(1, D),
        "w_risk": f16(params['W_risk']),
        "b_risk": f32(params['b_risk']).reshape(1, 1),
    }
    in_maps = []
    for c in range(n_cores):
        m = dict(shared)
        m["xT"] = meta['xT'][c]
        m["idx"] = meta['idx'][c]
        m["mask"] = meta['mask'][c]
        m["maskT"] = meta['maskT'][c]
        m["ea"] = meta['ea'][c]
        m["pool"] = meta['pool'][c]
        in_maps.append(m)
    return in_maps


PARAM_KEYS = ['W_pre', 'b_pre', 'W_src', 'W_dst', 'W_edge', 'att_src', 'att_dst',
              'att_edge', 'bias_conv', 'ln_gamma', 'ln_beta', 'prelu_a',
              'W_post1', 'b_post1', 'W_post2', 'b_post2', 'W_risk', 'b_risk']


def prepare(dbg=False, min_G=1, variant=None, **inputs):
    """Returns (nc, in_maps, G)."""
    params = {k: np.asarray(inputs[k]) for k in PARAM_KEYS}
    meta = build_meta(np.asarray(inputs['x']), np.asarray(inputs['edge_attr']),
                      np.asarray(inputs['edge_index']), np.asarray(inputs['batch']),
                      min_G=min_G)
    flags = _const_flags(params)
    nc = build_program(meta, flags, dbg=dbg, variant=variant)
    in_maps = build_in_maps(meta, params)
    return nc, in_maps, meta['G']


def kernel(**inputs):
    # the reference pools into G=25 graphs regardless of batch contents
    from concourse.bass_utils import run_bass_kernel_spmd
    nc, in_maps, G = prepare(min_G=25, **inputs)
    res = run_bass_kernel_spmd(nc, in_maps, core_ids=list(range(NCORES)))
    return np.asarray(res.results[0]["risk"], dtype=np.float32)


if __name__ == "__main__":
    pass

